# revision 9
# baseline (speedup 1.0000x reference)
"""Trainium2 Bass kernel for nn_Binary_module_44263932953138 (UCell/AMS gene-set
scoring module).

Sharding: genes split across 8 cores (B row-shard + x_rank/x_log2 column-shard,
weight replicated). Each core computes gs_c = B_c @ W locally, partial
R/bg_num/raw_num over its gene shard, then one AllReduce of the partial-sum
buffer; batchnorm + final projection computed (redundantly) on every core.

Self-contained: hardcodes shapes from the problem spec.
"""
import sys

for _p in ("/opt/trn_rl_repo", "/root/.axon_site/_ro/trn_rl_repo"):
    if _p not in sys.path:
        sys.path.insert(0, _p)

import numpy as np

import bass_rust
import concourse.bass as bass
import concourse.mybir as mybir
import concourse.tile as tile
from concourse.bass_utils import run_bass_kernel_spmd
from concourse.masks import make_identity

# ---------------------------------------------------------------------------
# Workaround for this container's walrus: every TPB instruction here accepts at
# most ONE sync-wait command, but Tile's sem assignment can attach several
# (e.g. the end-of-kernel drain, or a DMA waiting on multiple producers).
# Post-pass: hoist excess waits onto injected same-engine NoOps placed
# immediately before the instruction (the engine executes its stream in order,
# so wait-then-instruction semantics are preserved; for HWDGE DMAs this turns
# a queue-level wait into an issue-time wait, which is strictly stronger).
from concourse.tile import TileContext


def _split_multi_waits(nc, max_waits=1):
    for f in nc.m.functions:
        new_blocks = []
        for bb in f.blocks:
            rebuilt = []
            changed = False
            for ins in bb.instructions:
                si = ins.sync_info
                if si is not None and si.on_wait and len(si.on_wait) > max_waits:
                    waits = list(si.on_wait)
                    for w in waits[:-max_waits]:
                        nop = mybir.InstNoOp(
                            name=f"waitsplit-{nc.next_id()}", ins=[], outs=[]
                        )
                        nop.engine = ins.engine
                        nop.sync_info = bass_rust.SyncInfo(
                            on_wait=[w], on_update=[]
                        )
                        rebuilt.append(nop)
                    ins.sync_info = bass_rust.SyncInfo(
                        on_wait=waits[-max_waits:], on_update=list(si.on_update)
                    )
                    changed = True
                rebuilt.append(ins)
            if changed:
                nbb = bass_rust.BasicBlock(name=bb.name, instructions=rebuilt)
                nbb.IsExit = bb.IsExit
                nbb.IsLoopEntry = bb.IsLoopEntry
                nbb.IsPredicated = bb.IsPredicated
                new_blocks.append(nbb)
            else:
                new_blocks.append(bb)
        f.blocks = new_blocks
# ---------------------------------------------------------------------------

F32 = mybir.dt.float32
F32R = mybir.dt.float32r
N_CORES = 8
P = 128

# Full problem config
G_REAL = 14271     # real genes
H = 24             # gene sets
BATCH = 4096
EPS = 1e-5
MAXRANK_PARAM = 1000.0

# partial-sum buffer layout (partition rows; 32-aligned engine bases):
#   rows 0:24   R.T        (+ column `batch`..: gs_colsum as a column)
#   rows 32:56  bg_num.T
#   rows 64:88  raw_num.T
PR_ROWS = 88
PR_XCOL = 4  # extra columns; col[batch] holds gs_colsum


def build_nc(g_real=G_REAL, batch=BATCH, n_cores=N_CORES, nat_bufs=12):
    """Build the SPMD Bass program (identical on all cores; per-core data
    differs via inputs)."""
    gsh = -(-g_real // (n_cores * P)) * P        # per-core genes, mult of 128
    gp = gsh * n_cores                           # padded total genes
    GT = gsh // P                                # gene tiles per core
    JT = gp // P                                 # contraction tiles (all genes)
    assert batch % 512 == 0
    BC = batch // 512                            # batch chunks of 512
    JCW = min(14 * P, gp)                        # j-chunk width for B loads
    assert gp % JCW == 0
    NJC = gp // JCW
    JT_PER_CHUNK = JCW // P
    # g-groups of <=512 within the core's shard
    g_groups = []
    g0 = 0
    while g0 < gsh:
        ng = min(512, gsh - g0)
        g_groups.append((g0, ng))
        g0 += ng

    nc = bass.Bass(num_devices=n_cores)
    bsh = nc.declare_dram_parameter("bsh", [gsh, gp], F32, isOutput=False)
    xr = nc.declare_dram_parameter("xr", [batch, gsh], F32, isOutput=False)
    xl = nc.declare_dram_parameter("xl", [batch, gsh], F32, isOutput=False)
    wt = nc.declare_dram_parameter("wt", [gp, H], F32, isOutput=False)
    wtm = nc.declare_dram_parameter("wtm", [gsh, H], F32, isOutput=False)
    mrp = nc.declare_dram_parameter("mrp", [1, 1], F32, isOutput=False)
    ow = nc.declare_dram_parameter("ow", [1, 2 * H], F32, isOutput=False)
    ob = nc.declare_dram_parameter("ob", [1, 1], F32, isOutput=False)
    pred = nc.declare_dram_parameter("pred", [batch], F32, isOutput=True)

    PW = batch + PR_XCOL

    with TileContext(nc) as tc:
        with (
            tc.tile_pool(name="singles", bufs=1) as singles,
            tc.tile_pool(name="nat", bufs=nat_bufs) as nat,
            tc.tile_pool(name="tp", bufs=3) as tp,
            tc.tile_pool(name="small", bufs=2) as small,
            tc.tile_pool(name="dram", bufs=1, space="DRAM") as dram,
            tc.tile_pool(name="ptr", bufs=2, space="PSUM") as ptr,
        ):
            # ---------------- phase 0: prelude -----------------------------
            id128 = singles.tile([P, P], F32)
            make_identity(nc, id128)
            eps_sb = singles.tile([P, 1], F32)
            nc.vector.memset(eps_sb, EPS)
            ones_f32 = singles.tile([P, 1], F32)
            nc.vector.memset(ones_f32, 1.0)
            ones_sb = singles.tile([P, 1], F32R)
            nc.vector.tensor_copy(out=ones_sb, in_=ones_f32)

            # W (all genes), binarized:   [128, JT, 24]  (DMA into an f32
            # staging tile; fp32r tiles may only be written by rounding ops)
            w_stage = singles.tile([P, JT, H], F32)
            w_sb = singles.tile([P, JT, H], F32R)
            nc.sync.dma_start(out=w_stage,
                              in_=wt[:].rearrange("(t p) h -> p t h", p=P))
            nc.vector.tensor_scalar(
                out=w_sb[:], in0=w_stage[:], scalar1=0.0, scalar2=None,
                op0=mybir.AluOpType.is_gt,
            )
            # WG: per-core [gs | W_my] stationary  [128, GT, 48]
            wg_sb = singles.tile([P, GT, 2 * H], F32R)
            nc.sync.dma_start(
                out=w_stage[:, 0:GT, :],
                in_=wtm[:].rearrange("(t p) h -> p t h", p=P),
            )
            nc.vector.tensor_scalar(
                out=wg_sb[:, :, H:2 * H], in0=w_stage[:, 0:GT, :],
                scalar1=0.0, scalar2=None, op0=mybir.AluOpType.is_gt,
            )

            # n = W.sum(0) as a row [1, 24] via matmul with ones
            with tc.tile_pool(name="ps0", bufs=2, space="PSUM") as ps0:
                psum_n = ps0.tile([1, H], F32)
                for t in range(JT):
                    nc.tensor.matmul(
                        psum_n, lhsT=ones_sb, rhs=w_sb[:, t, :],
                        start=(t == 0), stop=(t == JT - 1),
                    )
                n_row = singles.tile([1, H], F32)
                nc.any.tensor_copy(out=n_row, in_=psum_n)

            nT = singles.tile([P, 1], F32)  # rows 0:24 = n per-partition
            nc.sync.dma_start(out=nT[0:H, 0:1], in_=n_row[0:1, 0:H])

            # maxrank = n.max() + 10 + max(mrp,0)*1000, broadcast to [128,1]
            mr1 = singles.tile([1, 4], F32)
            nc.vector.reduce_max(mr1[0:1, 0:1], n_row[0:1, :], axis=mybir.AxisListType.X)
            mrp_sb = singles.tile([1, 1], F32)
            nc.sync.dma_start(out=mrp_sb, in_=mrp[:])
            nc.vector.tensor_scalar(
                out=mr1[0:1, 1:2], in0=mrp_sb, scalar1=0.0, scalar2=MAXRANK_PARAM,
                op0=mybir.AluOpType.max, op1=mybir.AluOpType.mult,
            )
            nc.vector.tensor_tensor(
                out=mr1[0:1, 2:3], in0=mr1[0:1, 0:1], in1=mr1[0:1, 1:2],
                op=mybir.AluOpType.add,
            )
            nc.vector.tensor_scalar(
                out=mr1[0:1, 3:4], in0=mr1[0:1, 2:3], scalar1=10.0, scalar2=None,
                op0=mybir.AluOpType.add,
            )
            mr_dram = dram.tile([1, 1], F32)
            nc.sync.dma_start(out=mr_dram, in_=mr1[0:1, 3:4])
            mrb = singles.tile([P, 1], F32)
            nc.sync.dma_start(out=mrb, in_=mr_dram[:].to_broadcast((P, 1)))

            # partial-sums buffer (transposed layout), all-reduced later
            part_sb = singles.tile([PR_ROWS, PW], F32)
            nc.vector.memset(part_sb[:], 0.0)

            # ---------------- phase 1: gs_c = B_c @ W ----------------------
            with tc.tile_pool(name="ps1", bufs=2, space="PSUM") as ps1:
                for (gg0, ng) in g_groups:
                    gtiles = ng // P
                    psum_gsT = ps1.tile([H, 512], F32, tag="gsT", name="psum_gsT")[:, :ng]
                    jt_abs = 0
                    for jc in range(NJC):
                        bnats = []
                        for t in range(gtiles):
                            bn = nat.tile([P, JCW], F32, tag="nat", name="bn")
                            nc.sync.dma_start(
                                out=bn,
                                in_=bsh[gg0 + t * P: gg0 + (t + 1) * P,
                                        jc * JCW:(jc + 1) * JCW],
                            )
                            bnats.append(bn)
                        for jl in range(JT_PER_CHUNK):
                            bT = tp.tile([P, 512], F32R, tag="bT", name="bT")[:, :ng]
                            for t in range(gtiles):
                                ptile = ptr.tile([P, P], F32, tag="tr", name="ptile")
                                nc.tensor.transpose(
                                    ptile, bnats[t][:, jl * P:(jl + 1) * P], id128
                                )
                                nc.any.tensor_copy(
                                    out=bT[:, t * P:(t + 1) * P], in_=ptile
                                )
                            nc.tensor.matmul(
                                psum_gsT,
                                lhsT=w_sb[:, jt_abs, :],
                                rhs=bT,
                                start=(jt_abs == 0), stop=(jt_abs == JT - 1),
                            )
                            jt_abs += 1
                    # epilogue: gsT -> natural gs tiles into WG[:, :, 0:24]
                    gsT_sb = small.tile([H, 512], F32, tag="gsT_sb", name="gsT_sb")[:, :ng]
                    nc.any.tensor_copy(out=gsT_sb, in_=psum_gsT)
                    for t in range(gtiles):
                        ptile = ptr.tile([P, P], F32, tag="tr", name="ptile")
                        nc.tensor.transpose(
                            ptile[:, :H], gsT_sb[:, t * P:(t + 1) * P],
                            id128[0:H, 0:H],
                        )
                        nc.any.tensor_copy(
                            out=wg_sb[:, gg0 // P + t, 0:H], in_=ptile[:, :H]
                        )

            # ---------------- phase 2: partial R / bg / raw ----------------
            with tc.tile_pool(name="ps2", bufs=2, space="PSUM") as ps2:
                for bc in range(BC):
                    xr_nats, xl_nats = [], []
                    for t in range(4):
                        b0 = bc * 512 + t * P
                        xn = nat.tile([P, gsh], F32, tag="nat", name="xn")
                        nc.sync.dma_start(out=xn, in_=xr[b0:b0 + P, :])
                        xr_nats.append(xn)
                        yn = nat.tile([P, gsh], F32, tag="nat", name="yn")
                        nc.sync.dma_start(out=yn, in_=xl[b0:b0 + P, :])
                        xl_nats.append(yn)
                    psum_r = ps2.tile([H, 512], F32, tag="pr", name="psum_r")
                    psum_lg = ps2.tile([2 * H, 512], F32, tag="plg", name="psum_lg")
                    for gt in range(GT):
                        xrT = tp.tile([P, 512], F32R, tag="xrT", name="xrT")
                        xlT = tp.tile([P, 512], F32R, tag="xlT", name="xlT")
                        for t in range(4):
                            pa = ptr.tile([P, P], F32, tag="tr", name="pa")
                            nc.tensor.transpose(
                                pa, xr_nats[t][:, gt * P:(gt + 1) * P], id128
                            )
                            nc.any.tensor_scalar(
                                out=xrT[:, t * P:(t + 1) * P], in0=pa,
                                scalar1=mrb[:, 0:1], scalar2=None,
                                op0=mybir.AluOpType.min,
                            )
                            pb = ptr.tile([P, P], F32, tag="tr", name="pb")
                            nc.tensor.transpose(
                                pb, xl_nats[t][:, gt * P:(gt + 1) * P], id128
                            )
                            nc.any.tensor_copy(
                                out=xlT[:, t * P:(t + 1) * P], in_=pb
                            )
                        nc.tensor.matmul(
                            psum_r,
                            lhsT=wg_sb[:, gt, H:2 * H],
                            rhs=xrT,
                            start=(gt == 0), stop=(gt == GT - 1),
                        )
                        nc.tensor.matmul(
                            psum_lg,
                            lhsT=wg_sb[:, gt, :],
                            rhs=xlT,
                            start=(gt == 0), stop=(gt == GT - 1),
                        )
                    nc.any.tensor_copy(
                        out=part_sb[0:H, bc * 512:(bc + 1) * 512], in_=psum_r
                    )
                    stage48 = small.tile([2 * H, 512], F32, tag="stage48",
                                         name="stage48")
                    nc.any.tensor_copy(out=stage48, in_=psum_lg)
                    nc.sync.dma_start(
                        out=part_sb[32:56, bc * 512:(bc + 1) * 512],
                        in_=stage48[0:H, :],
                    )
                    nc.sync.dma_start(
                        out=part_sb[64:88, bc * 512:(bc + 1) * 512],
                        in_=stage48[H:2 * H, :],
                    )

            # gs column-sum partial -> extra column (via [1,24] -> [24,1] DMA)
            with tc.tile_pool(name="ps3", bufs=2, space="PSUM") as ps3:
                psum_cs = ps3.tile([1, H], F32, tag="cs", name="psum_cs")
                for gt in range(GT):
                    nc.tensor.matmul(
                        psum_cs, lhsT=ones_sb, rhs=wg_sb[:, gt, 0:H],
                        start=(gt == 0), stop=(gt == GT - 1),
                    )
                cs_row = small.tile([1, H], F32, tag="cs_row", name="cs_row")
                nc.any.tensor_copy(out=cs_row, in_=psum_cs)
                nc.sync.dma_start(
                    out=part_sb[0:H, batch:batch + 1], in_=cs_row[0:1, 0:H]
                )

                # ---------------- phase 3: all-reduce ----------------------
                cc_in = dram.tile([PR_ROWS, PW], F32)
                cc_out = dram.tile([PR_ROWS, PW], F32, addr_space="Shared")
                nc.sync.dma_start(out=cc_in, in_=part_sb)
                nc.gpsimd.collective_compute(
                    "AllReduce",
                    mybir.AluOpType.add,
                    replica_groups=[list(range(n_cores))],
                    ins=[cc_in[:]],
                    outs=[cc_out[:]],
                )
                sum_sb = singles.tile([PR_ROWS, PW], F32)
                nc.sync.dma_start(out=sum_sb, in_=cc_out)

                # ---------------- phase 4: final (redundant everywhere) ----
                # per-partition scalars, rows 0:24 (cols: 0 inv_n, 1 inv_nmr,
                # 2 s=-inv_nmr, 3 tconst, 4 inv_gs); rows 32:56 get shifted
                # copies (col 5 inv_n, col 6 inv_gs)
                sc = singles.tile([56, 8], F32)
                nc.vector.reciprocal(sc[0:H, 0:1], nT[0:H, 0:1])
                nc.vector.tensor_scalar(
                    out=sc[0:H, 1:2], in0=nT[0:H, 0:1], scalar1=mrb[0:H, 0:1],
                    scalar2=None, op0=mybir.AluOpType.mult,
                )
                nc.vector.reciprocal(sc[0:H, 1:2], sc[0:H, 1:2])
                nc.vector.tensor_scalar(
                    out=sc[0:H, 2:3], in0=sc[0:H, 1:2], scalar1=-1.0,
                    scalar2=None, op0=mybir.AluOpType.mult,
                )
                # tconst = 1 + n(n+1)/2 * inv_nmr
                nc.vector.tensor_scalar(
                    out=sc[0:H, 3:4], in0=nT[0:H, 0:1], scalar1=1.0,
                    scalar2=None, op0=mybir.AluOpType.add,
                )
                nc.vector.tensor_tensor(
                    out=sc[0:H, 3:4], in0=sc[0:H, 3:4], in1=nT[0:H, 0:1],
                    op=mybir.AluOpType.mult,
                )
                nc.vector.tensor_scalar(
                    out=sc[0:H, 3:4], in0=sc[0:H, 3:4],
                    scalar1=0.5, scalar2=sc[0:H, 1:2],
                    op0=mybir.AluOpType.mult, op1=mybir.AluOpType.mult,
                )
                nc.vector.tensor_scalar(
                    out=sc[0:H, 3:4], in0=sc[0:H, 3:4], scalar1=1.0,
                    scalar2=None, op0=mybir.AluOpType.add,
                )
                # inv_gs from the all-reduced colsum column
                nc.vector.reciprocal(sc[0:H, 4:5], sum_sb[0:H, batch:batch + 1])
                # shifted copies for the AMS block (rows 32:56)
                nc.sync.dma_start(out=sc[32:56, 5:6], in_=sc[0:H, 0:1])
                nc.sync.dma_start(out=sc[32:56, 6:7], in_=sc[0:H, 4:5])

                # R_all.T: rows 0:24 = UCell, rows 32:56 = AMS
                rall = singles.tile([56, batch], F32)
                nc.vector.tensor_scalar(
                    out=rall[0:H, :], in0=sum_sb[0:H, 0:batch],
                    scalar1=sc[0:H, 2:3], scalar2=sc[0:H, 3:4],
                    op0=mybir.AluOpType.mult, op1=mybir.AluOpType.add,
                )
                nc.sync.dma_start(
                    out=rall[32:56, :], in_=sum_sb[64:88, 0:batch]
                )
                nc.vector.tensor_scalar(
                    out=rall[32:56, :], in0=rall[32:56, :],
                    scalar1=sc[32:56, 5:6], scalar2=None,
                    op0=mybir.AluOpType.mult,
                )
                nc.vector.tensor_scalar(
                    out=sum_sb[32:56, 0:batch], in0=sum_sb[32:56, 0:batch],
                    scalar1=sc[32:56, 6:7], scalar2=None,
                    op0=mybir.AluOpType.mult,
                )
                nc.vector.tensor_tensor(
                    out=rall[32:56, :], in0=rall[32:56, :],
                    in1=sum_sb[32:56, 0:batch], op=mybir.AluOpType.subtract,
                )

                # batch-norm stats per 24-feature block
                nsub = batch // 512
                stats = small.tile([56, nsub, 6], F32, tag="bnstats", name="stats")
                mv = small.tile([56, 2], F32, tag="bnaggr", name="mv")
                rstd = small.tile([56, 1], F32, tag="rstd", name="rstd")
                for base in (0, 32):
                    blk = slice(base, base + H)
                    for s in range(nsub):
                        nc.vector.bn_stats(
                            out=stats[blk, s, :],
                            in_=rall[blk, s * 512:(s + 1) * 512],
                        )
                    nc.vector.bn_aggr(out=mv[blk], in_=stats[blk])
                    nc.scalar.activation(
                        out=rstd[blk], in_=mv[blk, 1:2],
                        func=mybir.ActivationFunctionType.Sqrt,
                        bias=eps_sb[blk], scale=1.0,
                    )
                    nc.vector.reciprocal(rstd[blk], rstd[blk])
                    nc.vector.tensor_scalar(
                        out=rall[blk, :], in0=rall[blk, :],
                        scalar1=mv[blk, 0:1], scalar2=rstd[blk],
                        op0=mybir.AluOpType.subtract, op1=mybir.AluOpType.mult,
                    )

                # pred = R_norm @ out_w.T + out_b
                ow_sb = small.tile([1, 2 * H], F32, tag="ow", name="ow_sb")
                nc.sync.dma_start(out=ow_sb, in_=ow[:])
                owT = small.tile([56, 1], F32, tag="owT", name="owT")
                nc.sync.dma_start(out=owT[0:H, 0:1], in_=ow_sb[0:1, 0:H])
                nc.sync.dma_start(out=owT[32:56, 0:1], in_=ow_sb[0:1, H:2 * H])
                ob_sb = small.tile([1, 1], F32, tag="ob", name="ob_sb")
                nc.sync.dma_start(out=ob_sb, in_=ob[:])
                for ch in range(batch // 512):
                    pp_u = ps3.tile([1, 512], F32, tag="ppu", name="pp_u")
                    pp_a = ps3.tile([1, 512], F32, tag="ppa", name="pp_a")
                    nc.tensor.matmul(
                        pp_u,
                        lhsT=owT[0:H, 0:1],
                        rhs=rall[0:H, ch * 512:(ch + 1) * 512],
                        start=True, stop=True,
                    )
                    nc.tensor.matmul(
                        pp_a,
                        lhsT=owT[32:56, 0:1],
                        rhs=rall[32:56, ch * 512:(ch + 1) * 512],
                        start=True, stop=True,
                    )
                    pout = small.tile([1, 512], F32, tag="pout", name="pout")
                    nc.vector.tensor_scalar(
                        out=pout, in0=pp_u, scalar1=ob_sb[0:1, 0:1],
                        scalar2=None, op0=mybir.AluOpType.add,
                    )
                    nc.vector.tensor_tensor(
                        out=pout, in0=pout, in1=pp_a, op=mybir.AluOpType.add
                    )
                    nc.sync.dma_start(
                        out=pred[ch * 512:(ch + 1) * 512], in_=pout[0:1, :]
                    )
    _split_multi_waits(nc)
    return nc


# ---------------------------------------------------------------------------
# host-side sharding + execution

_NC_CACHE = {}


def _get_nc(key, **kw):
    if key not in _NC_CACHE:
        _NC_CACHE[key] = build_nc(**kw)
    return _NC_CACHE[key]


def shard_inputs(x_rank, x_log2, B, weight, maxrank_p, out_w, out_b,
                 g_real=G_REAL, n_cores=N_CORES):
    gsh = -(-g_real // (n_cores * P)) * P
    gp = gsh * n_cores
    batch = x_rank.shape[0]
    f32 = np.float32

    wt_pad = np.zeros((gp, H), f32)
    wt_pad[:g_real] = weight
    mrp_in = np.asarray(maxrank_p, f32).reshape(1, 1)
    ow_in = np.asarray(out_w, f32).reshape(1, 2 * H)
    ob_in = np.asarray(out_b, f32).reshape(1, 1)

    in_maps = []
    for c in range(n_cores):
        lo = c * gsh
        hi = min((c + 1) * gsh, g_real)
        nre = max(0, hi - lo)
        bshc = np.zeros((gsh, gp), f32)
        if nre:
            bshc[:nre, :g_real] = B[lo:hi, :]
        xrc = np.zeros((batch, gsh), f32)
        xlc = np.zeros((batch, gsh), f32)
        if nre:
            xrc[:, :nre] = x_rank[:, lo:hi]
            xlc[:, :nre] = x_log2[:, lo:hi]
        in_maps.append({
            "bsh": bshc, "xr": xrc, "xl": xlc,
            "wt": wt_pad, "wtm": wt_pad[lo:lo + gsh],
            "mrp": mrp_in, "ow": ow_in, "ob": ob_in,
        })
    return in_maps


def kernel(x_rank, x_log2, B, weight, maxrank_p, out_w, out_b):
    nc = _get_nc("full")
    in_maps = shard_inputs(x_rank, x_log2, B, weight, maxrank_p, out_w, out_b)
    res = run_bass_kernel_spmd(nc, in_maps, core_ids=list(range(N_CORES)))
    return np.asarray(res.results[0]["pred"], np.float32).reshape(BATCH, 1)


# revision 20
# speedup vs baseline: 86.0545x; 86.0545x over previous
"""Trainium2 Bass kernel for nn_Binary_module_44263932953138 (UCell/AMS gene-set
scoring module).

Sharding: genes split across 8 cores (B row-shard + x_rank/x_log2 column-shard,
weight replicated). Each core computes gs_c = B_c @ W locally, partial
R/bg_num/raw_num over its gene shard, then one AllReduce of the partial-sum
buffer; batchnorm + final projection computed (redundantly) on every core.

Self-contained: hardcodes shapes from the problem spec.
"""
import sys

for _p in ("/opt/trn_rl_repo", "/root/.axon_site/_ro/trn_rl_repo"):
    if _p not in sys.path:
        sys.path.insert(0, _p)

import numpy as np

import bass_rust
import concourse.bass as bass
import concourse.mybir as mybir
import concourse.tile as tile
from concourse.bass_utils import run_bass_kernel_spmd
from concourse.masks import make_identity

# ---------------------------------------------------------------------------
# Workaround for this container's walrus: every TPB instruction here accepts at
# most ONE sync-wait command, but Tile's sem assignment can attach several
# (e.g. the end-of-kernel drain, or a DMA waiting on multiple producers).
# Post-pass: hoist excess waits onto injected same-engine NoOps placed
# immediately before the instruction (the engine executes its stream in order,
# so wait-then-instruction semantics are preserved; for HWDGE DMAs this turns
# a queue-level wait into an issue-time wait, which is strictly stronger).
from concourse.tile import TileContext


def _split_multi_waits(nc, max_waits=1):
    for f in nc.m.functions:
        new_blocks = []
        for bb in f.blocks:
            rebuilt = []
            changed = False
            for ins in bb.instructions:
                si = ins.sync_info
                if si is not None and si.on_wait and len(si.on_wait) > max_waits:
                    waits = list(si.on_wait)
                    for w in waits[:-max_waits]:
                        nop = mybir.InstNoOp(
                            name=f"waitsplit-{nc.next_id()}", ins=[], outs=[]
                        )
                        nop.engine = ins.engine
                        nop.sync_info = bass_rust.SyncInfo(
                            on_wait=[w], on_update=[]
                        )
                        rebuilt.append(nop)
                    ins.sync_info = bass_rust.SyncInfo(
                        on_wait=waits[-max_waits:], on_update=list(si.on_update)
                    )
                    changed = True
                rebuilt.append(ins)
            if changed:
                nbb = bass_rust.BasicBlock(name=bb.name, instructions=rebuilt)
                nbb.IsExit = bb.IsExit
                nbb.IsLoopEntry = bb.IsLoopEntry
                nbb.IsPredicated = bb.IsPredicated
                new_blocks.append(nbb)
            else:
                new_blocks.append(bb)
        f.blocks = new_blocks
# ---------------------------------------------------------------------------

F32 = mybir.dt.float32
F32R = mybir.dt.float32r
N_CORES = 8
P = 128

# Full problem config
G_REAL = 14271     # real genes
H = 24             # gene sets
BATCH = 4096
EPS = 1e-5
MAXRANK_PARAM = 1000.0

# partial-sum buffer layout (partition rows; 32-aligned engine bases):
#   rows 0:24   R.T        (+ column `batch`..: gs_colsum as a column)
#   rows 32:56  bg_num.T
#   rows 64:88  raw_num.T
PR_ROWS = 88
PR_XCOL = 4  # extra columns; col[batch] holds gs_colsum


def build_nc(g_real=G_REAL, batch=BATCH, n_cores=N_CORES, nat_bufs=12):
    """Build the SPMD Bass program (identical on all cores; per-core data
    differs via inputs)."""
    gsh = -(-g_real // (n_cores * P)) * P        # per-core genes, mult of 128
    gp = gsh * n_cores                           # padded total genes
    GT = gsh // P                                # gene tiles per core
    JT = gp // P                                 # contraction tiles (all genes)
    assert batch % 512 == 0
    BC = batch // 512                            # batch chunks of 512
    JCW = min(14 * P, gp)                        # j-chunk width for B loads
    assert gp % JCW == 0
    NJC = gp // JCW
    JT_PER_CHUNK = JCW // P
    # g-groups of <=512 within the core's shard
    g_groups = []
    g0 = 0
    while g0 < gsh:
        ng = min(512, gsh - g0)
        g_groups.append((g0, ng))
        g0 += ng

    nc = bass.Bass(num_devices=n_cores)
    bsh = nc.declare_dram_parameter("bsh", [gsh, gp], F32, isOutput=False)
    xr = nc.declare_dram_parameter("xr", [batch, gsh], F32, isOutput=False)
    xl = nc.declare_dram_parameter("xl", [batch, gsh], F32, isOutput=False)
    wt = nc.declare_dram_parameter("wt", [gp, H], F32, isOutput=False)
    wtm = nc.declare_dram_parameter("wtm", [gsh, H], F32, isOutput=False)
    mrp = nc.declare_dram_parameter("mrp", [1, 1], F32, isOutput=False)
    ow = nc.declare_dram_parameter("ow", [1, 2 * H], F32, isOutput=False)
    ob = nc.declare_dram_parameter("ob", [1, 1], F32, isOutput=False)
    pred = nc.declare_dram_parameter("pred", [batch], F32, isOutput=True)

    PW = batch + PR_XCOL

    with TileContext(nc) as tc:
        with (
            tc.tile_pool(name="singles", bufs=1) as singles,
            tc.tile_pool(name="nat", bufs=nat_bufs) as nat,
            tc.tile_pool(name="tp", bufs=4) as tp,
            tc.tile_pool(name="small", bufs=2) as small,
            tc.tile_pool(name="dram", bufs=1, space="DRAM") as dram,
            tc.tile_pool(name="ptr", bufs=4, space="PSUM") as ptr,
        ):
            # ---------------- phase 0: prelude -----------------------------
            id128 = singles.tile([P, P], F32)
            make_identity(nc, id128)
            eps_sb = singles.tile([P, 1], F32)
            nc.vector.memset(eps_sb, EPS)
            ones_f32 = singles.tile([P, 1], F32)
            nc.vector.memset(ones_f32, 1.0)
            ones_sb = singles.tile([P, 1], F32R)
            nc.vector.tensor_copy(out=ones_sb, in_=ones_f32)

            # W (all genes), binarized:   [128, JT, 24]  (DMA into an f32
            # staging tile; fp32r tiles may only be written by rounding ops)
            w_stage = singles.tile([P, JT, H], F32)
            w_sb = singles.tile([P, JT, H], F32R)
            nc.sync.dma_start(out=w_stage,
                              in_=wt[:].rearrange("(t p) h -> p t h", p=P))
            nc.vector.tensor_scalar(
                out=w_sb[:], in0=w_stage[:], scalar1=0.0, scalar2=None,
                op0=mybir.AluOpType.is_gt,
            )
            # WG: per-core [gs | W_my] stationary  [128, GT, 48]
            wg_sb = singles.tile([P, GT, 2 * H], F32R)
            nc.sync.dma_start(
                out=w_stage[:, 0:GT, :],
                in_=wtm[:].rearrange("(t p) h -> p t h", p=P),
            )
            nc.vector.tensor_scalar(
                out=wg_sb[:, :, H:2 * H], in0=w_stage[:, 0:GT, :],
                scalar1=0.0, scalar2=None, op0=mybir.AluOpType.is_gt,
            )

            # n = W.sum(0) as a row [1, 24] via matmul with ones
            with tc.tile_pool(name="ps0", bufs=2, space="PSUM") as ps0:
                psum_n = ps0.tile([1, H], F32)
                for t in range(JT):
                    nc.tensor.matmul(
                        psum_n, lhsT=ones_sb, rhs=w_sb[:, t, :],
                        start=(t == 0), stop=(t == JT - 1),
                    )
                n_row = singles.tile([1, H], F32)
                nc.any.tensor_copy(out=n_row, in_=psum_n)

            nT = singles.tile([P, 1], F32)  # rows 0:24 = n per-partition
            nc.sync.dma_start(out=nT[0:H, 0:1], in_=n_row[0:1, 0:H])

            # maxrank = n.max() + 10 + max(mrp,0)*1000, broadcast to [128,1]
            mr1 = singles.tile([1, 4], F32)
            nc.vector.reduce_max(mr1[0:1, 0:1], n_row[0:1, :], axis=mybir.AxisListType.X)
            mrp_sb = singles.tile([1, 1], F32)
            nc.sync.dma_start(out=mrp_sb, in_=mrp[:])
            nc.vector.tensor_scalar(
                out=mr1[0:1, 1:2], in0=mrp_sb, scalar1=0.0, scalar2=MAXRANK_PARAM,
                op0=mybir.AluOpType.max, op1=mybir.AluOpType.mult,
            )
            nc.vector.tensor_tensor(
                out=mr1[0:1, 2:3], in0=mr1[0:1, 0:1], in1=mr1[0:1, 1:2],
                op=mybir.AluOpType.add,
            )
            nc.vector.tensor_scalar(
                out=mr1[0:1, 3:4], in0=mr1[0:1, 2:3], scalar1=10.0, scalar2=None,
                op0=mybir.AluOpType.add,
            )
            mr_dram = dram.tile([1, 1], F32)
            nc.sync.dma_start(out=mr_dram, in_=mr1[0:1, 3:4])
            mrb = singles.tile([P, 1], F32)
            nc.sync.dma_start(out=mrb, in_=mr_dram[:].to_broadcast((P, 1)))

            # partial-sums buffer (transposed layout), all-reduced later
            part_sb = singles.tile([PR_ROWS, PW], F32)
            nc.vector.memset(part_sb[:], 0.0)

            # per-partition scalars, rows 0:24 (cols: 0 inv_n, 1 inv_nmr,
            # 2 s=-inv_nmr, 3 tconst, 4 inv_gs); rows 32:56 shifted copies
            # (col 5 inv_n, col 6 inv_gs). All except inv_gs computed now.
            sc = singles.tile([56, 8], F32)
            nc.vector.reciprocal(sc[0:H, 0:1], nT[0:H, 0:1])
            nc.vector.tensor_scalar(
                out=sc[0:H, 1:2], in0=nT[0:H, 0:1], scalar1=mrb[0:H, 0:1],
                scalar2=None, op0=mybir.AluOpType.mult,
            )
            nc.vector.reciprocal(sc[0:H, 1:2], sc[0:H, 1:2])
            nc.vector.tensor_scalar(
                out=sc[0:H, 2:3], in0=sc[0:H, 1:2], scalar1=-1.0,
                scalar2=None, op0=mybir.AluOpType.mult,
            )
            # tconst = 1 + n(n+1)/2 * inv_nmr
            nc.vector.tensor_scalar(
                out=sc[0:H, 3:4], in0=nT[0:H, 0:1], scalar1=1.0,
                scalar2=None, op0=mybir.AluOpType.add,
            )
            nc.vector.tensor_tensor(
                out=sc[0:H, 3:4], in0=sc[0:H, 3:4], in1=nT[0:H, 0:1],
                op=mybir.AluOpType.mult,
            )
            nc.vector.tensor_scalar(
                out=sc[0:H, 3:4], in0=sc[0:H, 3:4],
                scalar1=0.5, scalar2=sc[0:H, 1:2],
                op0=mybir.AluOpType.mult, op1=mybir.AluOpType.mult,
            )
            nc.vector.tensor_scalar(
                out=sc[0:H, 3:4], in0=sc[0:H, 3:4], scalar1=1.0,
                scalar2=None, op0=mybir.AluOpType.add,
            )
            nc.sync.dma_start(out=sc[32:56, 5:6], in_=sc[0:H, 0:1])

            # R_all.T: rows 0:24 = UCell, rows 32:56 = AMS.  Constructed
            # quarter-by-quarter as all-reduce results land.
            rall = singles.tile([56, batch], F32)
            nc.vector.memset(rall[:], 0.0)
            nsub = batch // 512
            stats = small.tile([56, nsub, 6], F32, tag="bnstats", name="stats")
            mv = small.tile([56, 2], F32, tag="bnaggr", name="mv")
            rstd = small.tile([56, 1], F32, tag="rstd", name="rstd")

            # ---------------- phase 1: gs_c = B_c @ W ----------------------
            with tc.tile_pool(name="ps1", bufs=2, space="PSUM") as ps1:
                for (gg0, ng) in g_groups:
                    gtiles = ng // P
                    psum_gsT = ps1.tile([H, 512], F32, tag="gsT", name="psum_gsT")[:, :ng]
                    jt_abs = 0
                    for jc in range(NJC):
                        bnats = []
                        for t in range(gtiles):
                            bn = nat.tile([P, JCW], F32, tag="nat", name="bn")
                            nc.sync.dma_start(
                                out=bn,
                                in_=bsh[gg0 + t * P: gg0 + (t + 1) * P,
                                        jc * JCW:(jc + 1) * JCW],
                            )
                            bnats.append(bn)
                        for jl in range(JT_PER_CHUNK):
                            bT = tp.tile([P, 512], F32R, tag="bT", name="bT")[:, :ng]
                            pt512 = ptr.tile([P, 512], F32, tag="tr", name="pt512")
                            for t in range(gtiles):
                                nc.tensor.matmul(
                                    pt512[:, t * P:(t + 1) * P],
                                    lhsT=bnats[t][:, jl * P:(jl + 1) * P],
                                    rhs=id128, is_transpose=True,
                                    start=(t == 0), stop=(t == gtiles - 1),
                                )
                            nc.any.tensor_copy(out=bT, in_=pt512[:, :ng])
                            nc.tensor.matmul(
                                psum_gsT,
                                lhsT=w_sb[:, jt_abs, :],
                                rhs=bT,
                                start=(jt_abs == 0), stop=(jt_abs == JT - 1),
                            )
                            jt_abs += 1
                    # epilogue: gsT -> natural gs tiles into WG[:, :, 0:24]
                    gsT_sb = small.tile([H, 512], F32, tag="gsT_sb", name="gsT_sb")[:, :ng]
                    nc.any.tensor_copy(out=gsT_sb, in_=psum_gsT)
                    for t in range(gtiles):
                        ptile = ptr.tile([P, P], F32, tag="tr", name="ptile")
                        nc.tensor.transpose(
                            ptile[:, :H], gsT_sb[:, t * P:(t + 1) * P],
                            id128[0:H, 0:H],
                        )
                        nc.any.tensor_copy(
                            out=wg_sb[:, gg0 // P + t, 0:H], in_=ptile[:, :H]
                        )

                # gs column-sum partial + tiny all-reduce (overlaps phase 2)
                psum_cs = ps1.tile([1, H], F32, tag="cs", name="psum_cs")
                for gt in range(GT):
                    nc.tensor.matmul(
                        psum_cs, lhsT=ones_sb, rhs=wg_sb[:, gt, 0:H],
                        start=(gt == 0), stop=(gt == GT - 1),
                    )
                cs_row = small.tile([1, H], F32, tag="cs_row", name="cs_row")
                nc.any.tensor_copy(out=cs_row, in_=psum_cs)
                cs_ci = dram.tile([1, H], F32, name="cs_ci")
                cs_co = dram.tile([1, H], F32, addr_space="Shared", name="cs_co")
                nc.sync.dma_start(out=cs_ci, in_=cs_row)
                nc.gpsimd.collective_compute(
                    "AllReduce", mybir.AluOpType.add,
                    replica_groups=[list(range(n_cores))],
                    ins=[cs_ci[:]], outs=[cs_co[:]],
                )
                csT = small.tile([H, 1], F32, tag="csT", name="csT")
                nc.sync.dma_start(out=csT, in_=cs_co[:])
                nc.vector.reciprocal(sc[0:H, 4:5], csT)
                nc.sync.dma_start(out=sc[32:56, 6:7], in_=sc[0:H, 4:5])

            # collective buffers (all-reduce runs in two halves, the first
            # overlapped with the second half of phase 2)
            sum_sb = singles.tile([PR_ROWS, PW], F32)
            # all-reduce in quarters overlapped with phase 2 (full config);
            # single chunk otherwise. list of (trigger_bc, lo, hi)
            if BC == 8:
                cc_splits = [(1, 0, 1024), (3, 1024, 2048), (5, 2048, 3072)]
                cc_last_lo = 3072
            else:
                cc_splits = []
                cc_last_lo = 0
            cc_bufs = {}
            for i, (_, lo, hi) in enumerate(cc_splits):
                cc_bufs[lo] = (
                    dram.tile([PR_ROWS, hi - lo], F32, name=f"cc_in{i}"),
                    dram.tile([PR_ROWS, hi - lo], F32, addr_space="Shared",
                              name=f"cc_out{i}"),
                )
            cc_bufs[cc_last_lo] = (
                dram.tile([PR_ROWS, PW - cc_last_lo], F32, name="cc_inL"),
                dram.tile([PR_ROWS, PW - cc_last_lo], F32, addr_space="Shared",
                          name="cc_outL"),
            )

            def emit_allreduce(lo, hi):
                ci, co = cc_bufs[lo]
                nc.sync.dma_start(out=ci[:], in_=part_sb[:, lo:hi])
                nc.gpsimd.collective_compute(
                    "AllReduce", mybir.AluOpType.add,
                    replica_groups=[list(range(n_cores))],
                    ins=[ci[:]], outs=[co[:]],
                )
                nc.sync.dma_start(out=sum_sb[:, lo:hi], in_=co[:])
                hib = min(hi, batch)
                if hib <= lo:
                    return
                # R_UCell = R*s + tconst
                nc.vector.tensor_scalar(
                    out=rall[0:H, lo:hib], in0=sum_sb[0:H, lo:hib],
                    scalar1=sc[0:H, 2:3], scalar2=sc[0:H, 3:4],
                    op0=mybir.AluOpType.mult, op1=mybir.AluOpType.add,
                )
                # R_AMS = raw*inv_n - bg*inv_gs
                nc.sync.dma_start(out=rall[32:56, lo:hib],
                                  in_=sum_sb[64:88, lo:hib])
                nc.vector.tensor_scalar(
                    out=rall[32:56, lo:hib], in0=rall[32:56, lo:hib],
                    scalar1=sc[32:56, 5:6], scalar2=None,
                    op0=mybir.AluOpType.mult,
                )
                nc.vector.tensor_scalar(
                    out=sum_sb[32:56, lo:hib], in0=sum_sb[32:56, lo:hib],
                    scalar1=sc[32:56, 6:7], scalar2=None,
                    op0=mybir.AluOpType.mult,
                )
                nc.vector.tensor_tensor(
                    out=rall[32:56, lo:hib], in0=rall[32:56, lo:hib],
                    in1=sum_sb[32:56, lo:hib], op=mybir.AluOpType.subtract,
                )
                for s in range(lo // 512, hib // 512):
                    nc.vector.bn_stats(
                        out=stats[:, s, :], in_=rall[:, s * 512:(s + 1) * 512]
                    )

            # ---------------- phase 2: partial R / bg / raw ----------------
            with tc.tile_pool(name="ps2", bufs=2, space="PSUM") as ps2:
                for bc in range(BC):
                    xr_nats, xl_nats = [], []
                    for t in range(4):
                        b0 = bc * 512 + t * P
                        xn = nat.tile([P, gsh], F32, tag="nat", name="xn")
                        nc.sync.dma_start(out=xn, in_=xr[b0:b0 + P, :])
                        xr_nats.append(xn)
                        yn = nat.tile([P, gsh], F32, tag="nat", name="yn")
                        nc.sync.dma_start(out=yn, in_=xl[b0:b0 + P, :])
                        xl_nats.append(yn)
                    psum_r = ps2.tile([H, 512], F32, tag="pr", name="psum_r")
                    psum_lg = ps2.tile([2 * H, 512], F32, tag="plg", name="psum_lg")
                    for gt in range(GT):
                        xrT = tp.tile([P, 512], F32R, tag="xrT", name="xrT")
                        xlT = tp.tile([P, 512], F32R, tag="xlT", name="xlT")
                        pa512 = ptr.tile([P, 512], F32, tag="tr", name="pa512")
                        pb512 = ptr.tile([P, 512], F32, tag="tr", name="pb512")
                        for t in range(4):
                            nc.tensor.matmul(
                                pa512[:, t * P:(t + 1) * P],
                                lhsT=xr_nats[t][:, gt * P:(gt + 1) * P],
                                rhs=id128, is_transpose=True,
                                start=(t == 0), stop=(t == 3),
                            )
                        for t in range(4):
                            nc.tensor.matmul(
                                pb512[:, t * P:(t + 1) * P],
                                lhsT=xl_nats[t][:, gt * P:(gt + 1) * P],
                                rhs=id128, is_transpose=True,
                                start=(t == 0), stop=(t == 3),
                            )
                        nc.any.tensor_scalar(
                            out=xrT, in0=pa512,
                            scalar1=mrb[:, 0:1], scalar2=None,
                            op0=mybir.AluOpType.min,
                        )
                        nc.any.tensor_copy(out=xlT, in_=pb512)
                        nc.tensor.matmul(
                            psum_r,
                            lhsT=wg_sb[:, gt, H:2 * H],
                            rhs=xrT,
                            start=(gt == 0), stop=(gt == GT - 1),
                        )
                        nc.tensor.matmul(
                            psum_lg,
                            lhsT=wg_sb[:, gt, :],
                            rhs=xlT,
                            start=(gt == 0), stop=(gt == GT - 1),
                        )
                    nc.any.tensor_copy(
                        out=part_sb[0:H, bc * 512:(bc + 1) * 512], in_=psum_r
                    )
                    stage48 = small.tile([2 * H, 512], F32, tag="stage48",
                                         name="stage48")
                    nc.any.tensor_copy(out=stage48, in_=psum_lg)
                    nc.sync.dma_start(
                        out=part_sb[32:56, bc * 512:(bc + 1) * 512],
                        in_=stage48[0:H, :],
                    )
                    nc.sync.dma_start(
                        out=part_sb[64:88, bc * 512:(bc + 1) * 512],
                        in_=stage48[H:2 * H, :],
                    )
                    for (trig, lo, hi) in cc_splits:
                        if bc == trig:
                            emit_allreduce(lo, hi)

            with tc.tile_pool(name="ps3", bufs=1, space="PSUM") as ps3:
                # ---------------- phase 3: all-reduce (last chunk) ---------
                emit_allreduce(cc_last_lo, PW)

                # ---------------- phase 4: final (redundant everywhere) ----
                nc.vector.bn_aggr(out=mv, in_=stats)
                nc.scalar.activation(
                    out=rstd, in_=mv[:, 1:2],
                    func=mybir.ActivationFunctionType.Sqrt,
                    bias=eps_sb[0:56], scale=1.0,
                )
                nc.vector.reciprocal(rstd, rstd)
                nc.vector.tensor_scalar(
                    out=rall[:], in0=rall[:],
                    scalar1=mv[:, 0:1], scalar2=rstd,
                    op0=mybir.AluOpType.subtract, op1=mybir.AluOpType.mult,
                )

                # pred = R_norm @ out_w.T + out_b
                ow_sb = small.tile([1, 2 * H], F32, tag="ow", name="ow_sb")
                nc.sync.dma_start(out=ow_sb, in_=ow[:])
                owT = small.tile([56, 1], F32, tag="owT", name="owT")
                nc.sync.dma_start(out=owT[0:H, 0:1], in_=ow_sb[0:1, 0:H])
                nc.sync.dma_start(out=owT[32:56, 0:1], in_=ow_sb[0:1, H:2 * H])
                ob_sb = small.tile([1, 1], F32, tag="ob", name="ob_sb")
                nc.sync.dma_start(out=ob_sb, in_=ob[:])
                for ch in range(batch // 512):
                    pp_u = ps3.tile([1, 512], F32, tag="ppu", name="pp_u")
                    pp_a = ps3.tile([1, 512], F32, tag="ppa", name="pp_a")
                    nc.tensor.matmul(
                        pp_u,
                        lhsT=owT[0:H, 0:1],
                        rhs=rall[0:H, ch * 512:(ch + 1) * 512],
                        start=True, stop=True,
                    )
                    nc.tensor.matmul(
                        pp_a,
                        lhsT=owT[32:56, 0:1],
                        rhs=rall[32:56, ch * 512:(ch + 1) * 512],
                        start=True, stop=True,
                    )
                    pout = small.tile([1, 512], F32, tag="pout", name="pout")
                    nc.vector.tensor_scalar(
                        out=pout, in0=pp_u, scalar1=ob_sb[0:1, 0:1],
                        scalar2=None, op0=mybir.AluOpType.add,
                    )
                    nc.vector.tensor_tensor(
                        out=pout, in0=pout, in1=pp_a, op=mybir.AluOpType.add
                    )
                    nc.sync.dma_start(
                        out=pred[ch * 512:(ch + 1) * 512], in_=pout[0:1, :]
                    )
    _split_multi_waits(nc)
    return nc


# ---------------------------------------------------------------------------
# host-side sharding + execution

_NC_CACHE = {}


def _get_nc(key, **kw):
    if key not in _NC_CACHE:
        _NC_CACHE[key] = build_nc(**kw)
    return _NC_CACHE[key]


def shard_inputs(x_rank, x_log2, B, weight, maxrank_p, out_w, out_b,
                 g_real=G_REAL, n_cores=N_CORES):
    gsh = -(-g_real // (n_cores * P)) * P
    gp = gsh * n_cores
    batch = x_rank.shape[0]
    f32 = np.float32

    wt_pad = np.zeros((gp, H), f32)
    wt_pad[:g_real] = weight
    mrp_in = np.asarray(maxrank_p, f32).reshape(1, 1)
    ow_in = np.asarray(out_w, f32).reshape(1, 2 * H)
    ob_in = np.asarray(out_b, f32).reshape(1, 1)

    in_maps = []
    for c in range(n_cores):
        lo = c * gsh
        hi = min((c + 1) * gsh, g_real)
        nre = max(0, hi - lo)
        bshc = np.zeros((gsh, gp), f32)
        if nre:
            bshc[:nre, :g_real] = B[lo:hi, :]
        xrc = np.zeros((batch, gsh), f32)
        xlc = np.zeros((batch, gsh), f32)
        if nre:
            xrc[:, :nre] = x_rank[:, lo:hi]
            xlc[:, :nre] = x_log2[:, lo:hi]
        in_maps.append({
            "bsh": bshc, "xr": xrc, "xl": xlc,
            "wt": wt_pad, "wtm": wt_pad[lo:lo + gsh],
            "mrp": mrp_in, "ow": ow_in, "ob": ob_in,
        })
    return in_maps


def kernel(x_rank, x_log2, B, weight, maxrank_p, out_w, out_b):
    nc = _get_nc("full")
    in_maps = shard_inputs(x_rank, x_log2, B, weight, maxrank_p, out_w, out_b)
    res = run_bass_kernel_spmd(nc, in_maps, core_ids=list(range(N_CORES)))
    return np.asarray(res.results[0]["pred"], np.float32).reshape(BATCH, 1)
